# revision 1
# baseline (speedup 1.0000x reference)
"""Causal linear attention (ELU+1 feature map) for Trainium2, 8 NeuronCores.

Sharding: core c handles batch b = c // 4 and head-group g = c % 4
(4 heads of 64 dims -> a 256-feature slice of the QKV/O projections).
Each core computes its partial O-projection output (2048, 1024) in bf16;
the host sums the 4 partials per batch in fp32 and adds bo.

v3: all-bf16 datapath. Hybrid chunked linear attention per 512-query
block: masked diagonal 128-chunk scores (PE + DVE mask), off-diagonal via
per-chunk KV prefix states. The prefix chain is ONE DVE tensor_tensor_scan
per (block, head): seed kv state lands in slot 0 of a [64, 65, 5] PSUM
tile via an identity matmul, the 4 chunk outer products land in slots
1..4, and scan(keep*state + data) with a 0/1 keep mask segments the
recurrence per v-column. den rides as row 64 of the [65, 512] num tile
(Vaug ones column); reciprocal_approx_fast + gpsimd partition_broadcast +
DVE mul produce out = num/den.

Engines: gpsimd never touches PSUM (HW rule). ACT: phi exp/relu, kn/v/y
evictions. DVE: phi min, masks, scan, recip, out mul, y n1 evictions.
Pool: phi add, rden broadcast. O-projection PSUM has its own pool so slow
y DMA readers never block attention PSUM rotation.
"""

import numpy as np
import ml_dtypes

import concourse.bacc as bacc
import concourse.bass as bass
import concourse.mybir as mybir
import concourse.tile as tile
from concourse.bass import ds, ts
from concourse.bass_utils import run_bass_kernel_spmd
from concourse.masks import make_identity, make_upper_triangular

B, S, H_DIM = 2, 2048, 1024
N_HEADS, HEAD_DIM = 16, 64
EPS = 1e-6

N_CORES = 8
HPC = 4                  # heads per core
O = HPC * HEAD_DIM       # 256: per-core projection feature slice
CH = 128                 # key chunk
QB = 512                 # query block
N_CH = S // CH           # 16
N_QB = S // QB           # 4
CPB = QB // CH           # 4 chunks per query block
KI = H_DIM // 128        # 8 contraction chunks
SB = 512                 # projection s-block width
N_SB = S // SB           # 4

FP32 = mybir.dt.float32
BF16 = mybir.dt.bfloat16

AF = mybir.ActivationFunctionType
ALU = mybir.AluOpType


DEBUG_DUMP = bool(int(__import__("os").environ.get("BASS_DBG", "0")))


def _emit(tc):
    nc = tc.nc
    xqT = nc.dram_tensor("xqT", [128, KI, S], BF16, kind="ExternalInput").ap()
    xkT = nc.dram_tensor("xkT", [128, KI, S], BF16, kind="ExternalInput").ap()
    xvT = nc.dram_tensor("xvT", [128, KI, S], BF16, kind="ExternalInput").ap()
    wq = nc.dram_tensor("wq", [128, KI, O], BF16, kind="ExternalInput").ap()
    wk = nc.dram_tensor("wk", [128, KI, O], BF16, kind="ExternalInput").ap()
    wv = nc.dram_tensor("wv", [128, KI, O], BF16, kind="ExternalInput").ap()
    wo = nc.dram_tensor("wo", [128, 2, H_DIM], BF16, kind="ExternalInput").ap()
    bqd = nc.dram_tensor("bq", [128, 2], FP32, kind="ExternalInput").ap()
    bkd = nc.dram_tensor("bk", [128, 2], FP32, kind="ExternalInput").ap()
    bvd = nc.dram_tensor("bv", [1, O], BF16, kind="ExternalInput").ap()
    y = nc.dram_tensor("y", [S, H_DIM], BF16, kind="ExternalOutput").ap()
    dbg = None
    if DEBUG_DUMP:
        dbg = {
            "qpt": nc.dram_tensor("d_qpt", [128, 2, S], BF16,
                                  kind="ExternalOutput").ap(),
            "kpt": nc.dram_tensor("d_kpt", [128, 2, S], BF16,
                                  kind="ExternalOutput").ap(),
            "vst": nc.dram_tensor("d_vst", [128, N_CH * HPC * 65], BF16,
                                  kind="ExternalOutput").ap(),
            "outt": nc.dram_tensor("d_outt", [128, 2, S], BF16,
                                   kind="ExternalOutput").ap(),
            "kvs": nc.dram_tensor("d_kvs", [128, N_QB, HPC, 65 * (CPB + 1)],
                                  BF16, kind="ExternalOutput").ap(),
            "sd": nc.dram_tensor("d_sd", [128, N_QB, HPC, QB], BF16,
                                 kind="ExternalOutput").ap(),
            "kn": nc.dram_tensor("d_kn", [128, N_QB, HPC, CPB * 64], BF16,
                                 kind="ExternalOutput").ap(),
        }

    with tc.tile_pool(name="singles", bufs=1) as singles:
        _emit_body(tc, singles, xqT, xkT, xvT, wq, wk, wv, wo, bqd, bkd, bvd,
                   y, dbg)


def _emit_body(tc, singles, xqT, xkT, xvT, wq, wk, wv, wo, bqd, bkd, bvd, y,
               dbg=None):
    nc = tc.nc
    # --- resident weights / constants -------------------------------------
    wq_s = singles.tile([128, KI, O], BF16, tag="wq")
    wk_s = singles.tile([128, KI, O], BF16, tag="wk")
    wv_s = singles.tile([128, KI, O], BF16, tag="wv")
    wo_s = singles.tile([128, 2, H_DIM], BF16, tag="wo")
    bq_s = singles.tile([128, 2], FP32, tag="bq")
    bk_s = singles.tile([128, 2], FP32, tag="bk")
    bv_s = singles.tile([1, O], BF16, tag="bv")

    ident = singles.tile([128, 64], BF16, tag="ident")
    make_identity(nc, ident[0:64, :])
    make_identity(nc, ident[64:128, :])
    ones = singles.tile([1, 128], BF16, tag="ones")
    nc.gpsimd.memset(ones[:], 1.0)
    umask4 = singles.tile([128, CPB * CH], BF16, tag="umask")
    make_upper_triangular(nc, umask4[:, 0:CH], val=1.0, diag=True)
    for cj in range(1, CPB):
        nc.gpsimd.tensor_copy(umask4[:, ts(cj, CH)], umask4[:, 0:CH])
    # keep mask for the KV prefix scan: 0 on seed slots, 1 elsewhere
    keep = singles.tile([64, 65, CPB + 1], BF16, tag="keep")
    nc.gpsimd.memset(keep[:], 1.0)
    nc.gpsimd.memset(keep[:, :, 0:1], 0.0)

    # --- resident activations ---------------------------------------------
    xq_s = singles.tile([128, KI, S], BF16, tag="xq")
    xk_s = singles.tile([128, KI, S], BF16, tag="xk")
    xv_s = singles.tile([128, KI, S], BF16, tag="xv")
    qpt = [singles.tile([128, S], BF16, tag=f"qpt{m}", name=f"qpt{m}") for m in range(2)]
    kpt = [singles.tile([128, S], BF16, tag=f"kpt{m}", name=f"kpt{m}") for m in range(2)]
    vst = singles.tile([128, N_CH, HPC, 65], BF16, tag="vst")
    nc.gpsimd.memset(vst[:, :, :, 64:65], 1.0)
    outt = [singles.tile([128, S], BF16, tag=f"outt{c}", name=f"outt{c}") for c in range(2)]
    kv0 = singles.tile([128, 65], BF16, tag="kv0")
    nc.gpsimd.memset(kv0[:], 0.0)

    # per-head running KV prefix state (AP into kvs_pool tiles after block 0)
    kv_cur = {h: kv0[ds(64 * (h % 2), 64), :] for h in range(HPC)}

    with (
        tc.tile_pool(name="phi", bufs=4) as phi_pool,
        tc.tile_pool(name="ssb", bufs=4) as ssb_pool,
        tc.tile_pool(name="kvs", bufs=2) as kvs_pool,
        tc.tile_pool(name="den", bufs=4) as den_pool,
        tc.tile_pool(name="yt", bufs=4) as yt_pool,
    ):

        def prefetch_x(sb, split=False):
            scol = ds(sb * SB, SB)
            if split:
                nc.sync.dma_start(xq_s[:, 0:4, scol], xqT[:, 0:4, scol])
                nc.sync.dma_start(xq_s[:, 4:8, scol], xqT[:, 4:8, scol])
                nc.sync.dma_start(xv_s[:, 0:4, scol], xvT[:, 0:4, scol])
                nc.sync.dma_start(xv_s[:, 4:8, scol], xvT[:, 4:8, scol])
            else:
                nc.sync.dma_start(xq_s[:, :, scol], xqT[:, :, scol])
                nc.sync.dma_start(xv_s[:, :, scol], xvT[:, :, scol])
            nc.sync.dma_start(xk_s[:, :, scol], xkT[:, :, scol])

        def phi_evict(p_x, b_x, m, dst, scol, pref, add_eng=None):
            e_t = phi_pool.tile([128, SB], BF16, tag="e", name=f"e_{pref}")
            nc.scalar.activation(e_t[:], p_x[:], AF.Exp, bias=b_x[:, ds(m, 1)])
            r_t = phi_pool.tile([128, SB], BF16, tag="r", name=f"r_{pref}")
            nc.scalar.activation(r_t[:], p_x[:], AF.Relu, bias=b_x[:, ds(m, 1)])
            nc.vector.tensor_scalar_min(e_t[:], e_t[:], 1.0)
            (add_eng or nc.gpsimd).tensor_add(dst[:, scol], r_t[:], e_t[:])

        def emit_A_q(sb, pqk_pool):
            scol = ds(sb * SB, SB)
            p_q = [pqk_pool.tile([128, SB], FP32, tag=f"m{m}", name=f"pq{m}_{sb}")
                   for m in range(2)]
            for ic in range(KI):
                for m in range(2):
                    nc.tensor.matmul(
                        p_q[m][:], wq_s[:, ic, ts(m, 128)], xq_s[:, ic, scol],
                        start=(ic == 0), stop=(ic == KI - 1),
                    )
            for m in range(2):
                phi_evict(p_q[m], bq_s, m, qpt[m], scol, f"q{m}_{sb}")

        def emit_A_v(sb, pv_pool):
            p_v = [pv_pool.tile([128, SB], FP32, tag=f"v{i}", name=f"pv{i}_{sb}")
                   for i in range(2)]
            for ic in range(KI):
                for st in range(4):
                    nc.tensor.matmul(
                        p_v[st // 2][:, ds((st % 2) * O, O)],
                        xv_s[:, ic, ds(sb * SB + st * 128, 128)], wv_s[:, ic, :],
                        start=(ic == 0 and st % 2 == 0), stop=False,
                    )
            for st in range(4):
                nc.tensor.matmul(
                    p_v[st // 2][:, ds((st % 2) * O, O)], ones[:, 0:128], bv_s[:],
                    start=False, stop=(st % 2 == 1),
                )
            for st in range(4):
                c = sb * 4 + st
                nc.scalar.copy(
                    vst[:, c, :, 0:64],
                    p_v[st // 2][:, ds((st % 2) * O, O)].rearrange(
                        "p (h d) -> p h d", h=HPC),
                )

        def emit_A_qv0(pqk_pool, pv_pool):
            """Block-0 q+v projections interleaved per-ic to ride the
            per-ic DMA arrivals at kernel start."""
            scol = ds(0, SB)
            p_q = [pqk_pool.tile([128, SB], FP32, tag=f"m{m}", name=f"pq{m}_0")
                   for m in range(2)]
            p_v = [pv_pool.tile([128, SB], FP32, tag=f"v{i}", name=f"pv{i}_0")
                   for i in range(2)]
            for ic in range(KI):
                for m in range(2):
                    nc.tensor.matmul(
                        p_q[m][:], wq_s[:, ic, ts(m, 128)], xq_s[:, ic, scol],
                        start=(ic == 0), stop=(ic == KI - 1),
                    )
                for st in range(4):
                    nc.tensor.matmul(
                        p_v[st // 2][:, ds((st % 2) * O, O)],
                        xv_s[:, ic, ds(st * 128, 128)], wv_s[:, ic, :],
                        start=(ic == 0 and st % 2 == 0), stop=False,
                    )
            for m in range(2):
                phi_evict(p_q[m], bq_s, m, qpt[m], scol, f"q{m}_0")
            for st in range(4):
                nc.tensor.matmul(
                    p_v[st // 2][:, ds((st % 2) * O, O)], ones[:, 0:128], bv_s[:],
                    start=False, stop=(st % 2 == 1),
                )
            for st in range(4):
                nc.scalar.copy(
                    vst[:, st, :, 0:64],
                    p_v[st // 2][:, ds((st % 2) * O, O)].rearrange(
                        "p (h d) -> p h d", h=HPC),
                )

        def emit_A_k(sb, pqk_pool):
            scol = ds(sb * SB, SB)
            p_k = [pqk_pool.tile([128, SB], FP32, tag=f"m{m}", name=f"pk{m}_{sb}")
                   for m in range(2)]
            for ic in range(KI):
                for m in range(2):
                    nc.tensor.matmul(
                        p_k[m][:], wk_s[:, ic, ts(m, 128)], xk_s[:, ic, scol],
                        start=(ic == 0), stop=(ic == KI - 1),
                    )
            for m in range(2):
                phi_evict(p_k[m], bk_s, m, kpt[m], scol, f"k{m}_{sb}", add_eng=nc.vector)

        def attn_qk(qb, heads, pnum_pool, pbig_pool):
            """Scores/masks/transposes for `heads` of block qb (no vst dep)."""
            ctx = {}
            for h in heads:
                mt, prow = h // 2, 64 * (h % 2)
                ctx[h] = dict(
                    mt=mt, prow=prow,
                    qp=qpt[mt][ds(prow, 64), :],
                    kp=kpt[mt][ds(prow, 64), :],
                    p_num=pnum_pool.tile([65, QB], FP32, tag="num",
                                         name=f"num{qb}_{h}"),
                )
            # diagonal scores (masked) + transposes
            for h in heads:
                x = ctx[h]
                p_sd = pbig_pool.tile([128, QB], FP32, tag="big",
                                      name=f"psd{qb}_{h}")
                for cj in range(CPB):
                    c = qb * CPB + cj
                    nc.tensor.matmul(
                        p_sd[:, ts(cj, CH)], x["kp"][:, ds(c * CH, CH)],
                        x["qp"][:, ds(qb * QB + cj * CH, CH)],
                        start=(cj == 0), stop=(cj == CPB - 1),
                    )
                x["p_sd"] = p_sd
            for h in heads:
                x = ctx[h]
                sd_t = ssb_pool.tile([128, QB], BF16, tag="sd",
                                     name=f"sd{qb}_{h}")
                nc.vector.tensor_mul(sd_t[:], x["p_sd"][:], umask4[:])
                x["sd_t"] = sd_t
            for h in heads:
                x = ctx[h]
                p_kn = pbig_pool.tile([128, CPB, 64], BF16, tag="big",
                                      name=f"pkn{qb}_{h}")
                for cj in range(CPB):
                    c = qb * CPB + cj
                    nc.tensor.matmul(
                        p_kn[:, cj, :], x["kp"][:, ds(c * CH, CH)],
                        ident[ds(x["prow"], 64), :], is_transpose=True,
                        start=(cj == 0), stop=(cj == CPB - 1),
                    )
                x["p_kn"] = p_kn
            for h in heads:
                x = ctx[h]
                kn_t = ssb_pool.tile([128, CPB, 64], BF16, tag="kn",
                                     name=f"kn{qb}_{h}")
                nc.scalar.copy(kn_t[:], x["p_kn"][:])
                x["kn_t"] = kn_t
            ctx["_qb"], ctx["_heads"] = qb, heads
            ctx["_pbig"] = pbig_pool
            return ctx

        def attn_vst(ctx):
            """num-diag + per-chunk KV outer products (needs vst)."""
            qb, heads = ctx["_qb"], ctx["_heads"]
            pbig_pool = ctx["_pbig"]
            # num: diagonal contributions
            for h in heads:
                x = ctx[h]
                for cj in range(CPB):
                    c = qb * CPB + cj
                    nc.tensor.matmul(
                        x["p_num"][:, ts(cj, CH)], vst[:, c, h, :],
                        x["sd_t"][:, ts(cj, CH)],
                        start=(cj == 0), stop=False,
                    )
            # KV prefix tile: slot 0 seeded with kv state via identity MM,
            # slots 1..4 = per-chunk outer products (independent groups)
            for h in heads:
                x = ctx[h]
                p_kv5 = pbig_pool.tile([64, 65, CPB + 1], FP32, tag="big",
                                       name=f"pkv{qb}_{h}")
                nc.tensor.matmul(
                    p_kv5[:, :, 0], ident[ds(x["prow"], 64), :], kv_cur[h],
                    start=True, stop=False,
                )
                for cj in range(CPB):
                    c = qb * CPB + cj
                    nc.tensor.matmul(
                        p_kv5[:, :, cj + 1], x["kn_t"][:, cj, :],
                        vst[:, c, h, :], start=False, stop=(cj == CPB - 1),
                    )
                x["p_kv5"] = p_kv5
            return ctx

        def attn_front(qb, heads, pnum_pool, pbig_pool):
            return attn_vst(attn_qk(qb, heads, pnum_pool, pbig_pool))

        def attn_back(ctx, fast_den=False, split_mul=False):
            """Scan/inter/den for a pair whose front is already emitted."""
            qb, heads = ctx["_qb"], ctx["_heads"]
            qcol = ds(qb * QB, QB)
            # one scan per head: state = keep*state + p_kv5 (segmented per
            # v-column) -> prefix states land in SBUF at prow
            for h in heads:
                x = ctx[h]
                kvs_t = kvs_pool.tile([128, 65, CPB + 1], BF16, tag=f"kvp{h}",
                                      name=f"kvp{qb}_{h}")
                kvs = kvs_t[ds(x["prow"], 64), :, :]
                nc.vector.tensor_tensor_scan(
                    kvs.rearrange("p a b -> p (a b)"),
                    keep[:].rearrange("p a b -> p (a b)"),
                    x["p_kv5"][:].rearrange("p a b -> p (a b)"),
                    0.0, ALU.mult, ALU.add,
                )
                x["kvs"] = kvs
                if dbg is not None:
                    nc.sync.dma_start(
                        dbg["kvs"][:, qb, h, :],
                        kvs_t[:].rearrange("p a b -> p (a b)"))
                    nc.sync.dma_start(
                        dbg["sd"][:, qb, h, :], x["sd_t"][:])
                    nc.sync.dma_start(
                        dbg["kn"][:, qb, h, :],
                        x["kn_t"][:].rearrange("p a b -> p (a b)"))
            # num: off-diagonal/inter contributions via prefix states
            for h in heads:
                x = ctx[h]
                for cj in range(CPB):
                    if qb == 0 and cj == 0:
                        continue
                    nc.tensor.matmul(
                        x["p_num"][:, ts(cj, CH)], x["kvs"][:, :, cj],
                        x["qp"][:, ds(qb * QB + cj * CH, CH)],
                        start=False, stop=(cj == CPB - 1),
                    )
                kv_cur[h] = x["kvs"][:, :, CPB]
            # den + out
            for h in heads:
                x = ctx[h]
                den_t = den_pool.tile([1, QB], FP32, tag="den",
                                      name=f"den{qb}_{h}")
                if fast_den:  # same-queue copy: no ACT->DVE sem hop
                    nc.vector.tensor_copy(den_t[:], x["p_num"][ds(64, 1), :])
                else:
                    nc.scalar.copy(den_t[:], x["p_num"][ds(64, 1), :])
                rden = den_pool.tile([1, QB], FP32, tag="rden",
                                     name=f"rden{qb}_{h}")
                nc.vector.reciprocal_approx_fast(rden[:], den_t[:])
                bc = den_pool.tile([64, QB], FP32, tag="bc",
                                   name=f"bc{qb}_{h}")
                nc.gpsimd.partition_broadcast(bc[:], rden[:])
                if split_mul:  # let the first tail C group start earlier
                    h2 = QB // 2
                    nc.vector.tensor_mul(
                        outt[x["mt"]][ds(x["prow"], 64), ds(qb * QB, h2)],
                        x["p_num"][0:64, 0:h2], bc[:, 0:h2],
                    )
                    nc.vector.tensor_mul(
                        outt[x["mt"]][ds(x["prow"], 64), ds(qb * QB + h2, h2)],
                        x["p_num"][0:64, h2:QB], bc[:, h2:QB],
                    )
                else:
                    nc.vector.tensor_mul(
                        outt[x["mt"]][ds(x["prow"], 64), qcol],
                        x["p_num"][0:64, :], bc[:],
                    )

        def emit_C(sts, pc_pool, dma_eng=None, all_act=False):
            nst = len(sts)
            st0 = sts[0]
            y_t = yt_pool.tile([128, nst, H_DIM], BF16, tag="y",
                               name=f"yt{st0}")
            for i, st in enumerate(sts):
                for n in range(2):
                    p_o = pc_pool.tile([128, 512], FP32, tag="big",
                                       name=f"po{st}_{n}")
                    for ct in range(2):
                        nc.tensor.matmul(
                            p_o[:], outt[ct][:, ts(st, 128)],
                            wo_s[:, ct, ts(n, 512)],
                            start=(ct == 0), stop=(ct == 1),
                        )
                    if n == 0 or all_act:
                        nc.scalar.copy(y_t[:, i, ts(n, 512)], p_o[:])
                    else:
                        nc.vector.tensor_copy(y_t[:, i, ts(n, 512)], p_o[:])
            (dma_eng or nc.sync).dma_start(
                y[ds(st0 * 128, nst * 128), :].rearrange(
                    "(i p) c -> p i c", p=128),
                y_t[:],
            )

        # ---- emission schedule -------------------------------------------
        with tc.tile_pool(name="pnum", bufs=2, space="PSUM") as pnum_e:
            with (
                tc.tile_pool(name="pqk", bufs=1, space="PSUM") as pqk_pool,
                tc.tile_pool(name="pv", bufs=1, space="PSUM") as pv_pool,
                tc.tile_pool(name="pbig", bufs=2, space="PSUM") as pbig_e,
            ):
                # DMA order: first A's operands first, then stay a stage ahead.
                nc.sync.dma_start(wq_s[:, 0:4, :], wq[:, 0:4, :])
                nc.sync.dma_start(xq_s[:, 0:4, 0:SB], xqT[:, 0:4, 0:SB])
                nc.sync.dma_start(wq_s[:, 4:8, :], wq[:, 4:8, :])
                nc.sync.dma_start(xq_s[:, 4:8, 0:SB], xqT[:, 4:8, 0:SB])
                nc.sync.dma_start(bq_s[:], bqd)
                nc.sync.dma_start(wv_s[:], wv)
                nc.sync.dma_start(xv_s[:, 0:4, 0:SB], xvT[:, 0:4, 0:SB])
                nc.sync.dma_start(xv_s[:, 4:8, 0:SB], xvT[:, 4:8, 0:SB])
                nc.sync.dma_start(bv_s[:], bvd)
                nc.sync.dma_start(wk_s[:], wk)
                nc.sync.dma_start(xk_s[:, 0:4, 0:SB], xkT[:, 0:4, 0:SB])
                nc.sync.dma_start(xk_s[:, 4:8, 0:SB], xkT[:, 4:8, 0:SB])
                nc.sync.dma_start(bk_s[:], bkd)
                emit_A_q(0, pqk_pool)
                scol1 = ds(SB, SB)
                nc.sync.dma_start(xv_s[:, :, scol1], xvT[:, :, scol1])
                nc.sync.dma_start(xq_s[:, :, scol1], xqT[:, :, scol1])
                nc.sync.dma_start(xk_s[:, :, scol1], xkT[:, :, scol1])
                emit_A_v(0, pv_pool)
                emit_A_k(0, pqk_pool)
                emit_A_v(1, pv_pool)
                emit_A_q(1, pqk_pool)
                f = attn_front(0, [0, 1], pnum_e, pbig_e)
                attn_back(f)
                f = attn_front(0, [2, 3], pnum_e, pbig_e)
                emit_A_k(1, pqk_pool)
                attn_back(f)
                prefetch_x(2)
                nc.sync.dma_start(wo_s[:], wo)
                emit_A_q(2, pqk_pool)
                f = attn_front(1, [0, 1], pnum_e, pbig_e)
                emit_A_v(2, pv_pool)
                attn_back(f)
                f = attn_front(1, [2, 3], pnum_e, pbig_e)
                emit_A_k(2, pqk_pool)
                attn_back(f)
                prefetch_x(3)
                emit_A_q(3, pqk_pool)
                f = attn_front(2, [0, 1], pnum_e, pbig_e)
                emit_A_v(3, pv_pool)
                attn_back(f)
                f = attn_front(2, [2, 3], pnum_e, pbig_e)
                emit_A_k(3, pqk_pool)
                attn_back(f)
            with (
                tc.tile_pool(name="ptail", bufs=3, space="PSUM") as ptail,
                tc.tile_pool(name="pc", bufs=3, space="PSUM") as pc_pool,
            ):
                f = attn_front(3, [0, 1], pnum_e, ptail)
                emit_C([0, 1], pc_pool)
                attn_back(f)
                f = attn_front(3, [2, 3], pnum_e, ptail)
                emit_C([2, 3], pc_pool, all_act=True)
                attn_back(f, split_mul=True)
                emit_C([4, 5], pc_pool, all_act=True)
                emit_C([6, 7], pc_pool)
                emit_C([8, 9], pc_pool)
                emit_C([10, 11], pc_pool)
                emit_C([12, 13], pc_pool)
                emit_C([14], pc_pool)
                emit_C([15], pc_pool)
                if dbg is not None:
                    for m in range(2):
                        nc.sync.dma_start(dbg["qpt"][:, m, :], qpt[m][:])
                        nc.sync.dma_start(dbg["kpt"][:, m, :], kpt[m][:])
                        nc.sync.dma_start(dbg["outt"][:, m, :], outt[m][:])
                    nc.sync.dma_start(
                        dbg["vst"][:],
                        vst[:].rearrange("p a b c -> p (a b c)"))


_PROGRAM = None


def _get_program():
    global _PROGRAM
    if _PROGRAM is None:
        nc = bacc.Bacc("TRN2", target_bir_lowering=False, debug=False)
        with tile.TileContext(nc) as tc:
            _emit(tc)
        nc.compile()
        _PROGRAM = nc
    return _PROGRAM


def make_in_maps(inputs):
    BF = ml_dtypes.bfloat16
    query, key, value = (np.asarray(inputs[k], np.float32)
                         for k in ("query", "key", "value"))
    Wq, Wk, Wv, Wo = (np.asarray(inputs[k], np.float32)
                      for k in ("Wq", "Wk", "Wv", "Wo"))
    bq, bk, bv = (np.asarray(inputs[k], np.float32) for k in ("bq", "bk", "bv"))

    def xprep(x, b):  # (2048, 1024) -> (128, 8, 2048) bf16
        return np.ascontiguousarray(
            x[b].T.reshape(KI, 128, S).transpose(1, 0, 2)
        ).astype(BF)

    def wslice(W, g):  # (1024, 256)-slice -> (128, 8, 256) bf16
        wt = W[g * O:(g + 1) * O].T  # (1024, 256)
        return np.ascontiguousarray(
            wt.reshape(KI, 128, O).transpose(1, 0, 2)
        ).astype(BF)

    in_maps = []
    for c in range(N_CORES):
        b, g = divmod(c, 4)
        sl = slice(g * O, (g + 1) * O)
        in_maps.append({
            "xqT": xprep(query, b),
            "xkT": xprep(key, b),
            "xvT": xprep(value, b),
            "wq": wslice(Wq, g),
            "wk": wslice(Wk, g),
            "wv": wslice(Wv, g),
            "wo": np.ascontiguousarray(
                Wo[:, sl].T.reshape(2, 128, H_DIM).transpose(1, 0, 2)
            ).astype(BF),
            "bq": np.ascontiguousarray(bq[sl].reshape(2, 128).T),
            "bk": np.ascontiguousarray(bk[sl].reshape(2, 128).T),
            "bv": np.ascontiguousarray(bv[sl].reshape(1, O)).astype(BF),
        })
    return in_maps


def kernel(query, key, value, Wq, bq, Wk, bk, Wv, bv, Wo, bo, _trace=False):
    bo = np.asarray(bo, np.float32)
    in_maps = make_in_maps(dict(
        query=query, key=key, value=value, Wq=Wq, Wk=Wk, Wv=Wv, Wo=Wo,
        bq=bq, bk=bk, bv=bv,
    ))

    nc = _get_program()
    res = run_bass_kernel_spmd(
        nc, in_maps, core_ids=list(range(N_CORES)), trace=_trace
    )
    out = np.empty((B, S, H_DIM), np.float32)
    for b in range(B):
        acc = res.results[4 * b]["y"].astype(np.float32)
        for g in range(1, 4):
            acc += res.results[4 * b + g]["y"].astype(np.float32)
        out[b] = acc + bo
    if _trace:
        kernel.last_result = res
    return out



# revision 20
# speedup vs baseline: 1.2563x; 1.2563x over previous
"""Causal linear attention (ELU+1 feature map) for Trainium2, 8 NeuronCores.

Sharding: core c handles batch b = c // 4 and head-group g = c % 4
(4 heads of 64 dims -> a 256-feature slice of the QKV/O projections).
Each core computes its partial O-projection output (2048, 1024) in bf16;
the host sums the 4 partials per batch in fp32 and adds bo.

v4: software-pipelined attention. Per 512-query block: masked diagonal
128-chunk scores (PE + DVE mask), off-diagonal via per-chunk KV prefix
states built with one DVE tensor_tensor_scan per (block, head). front()
emits scores/masks/transposes/num-diag/kv-outers/scan; back_pe() emits
the num-inter matmuls one pair later so the PE queue never waits on the
scan. The den reciprocal reads the PSUM num row directly (DVE), is
broadcast to 64 partitions by a sync-queue SBUF->SBUF DMA (partition
stride 0), and the out = num * rden mul is deferred one more pair so the
DVE queue never waits on the broadcast DMA. gpsimd only does startup
memsets and the q-path phi adds. O-projection + y DMA are spread across
blocks (emit_C right after each block's attention) instead of all at the
tail; y DMAs fire from the sync queue.

x DRAM layout is [128, N_SB, KI, SB] so each 512-column block transfer
is one contiguous 8KB run per partition.
"""

import numpy as np
import ml_dtypes

import concourse.bacc as bacc
import concourse.bass as bass
import concourse.mybir as mybir
import concourse.tile as tile
from concourse.bass import ds, ts
from concourse.bass_utils import run_bass_kernel_spmd
from concourse.masks import make_identity, make_upper_triangular

B, S, H_DIM = 2, 2048, 1024
N_HEADS, HEAD_DIM = 16, 64
EPS = 1e-6

N_CORES = 8
HPC = 4                  # heads per core
O = HPC * HEAD_DIM       # 256: per-core projection feature slice
CH = 128                 # key chunk
QB = 512                 # query block
N_CH = S // CH           # 16
N_QB = S // QB           # 4
CPB = QB // CH           # 4 chunks per query block
KI = H_DIM // 128        # 8 contraction chunks
SB = 512                 # projection s-block width
N_SB = S // SB           # 4

FP32 = mybir.dt.float32
BF16 = mybir.dt.bfloat16

AF = mybir.ActivationFunctionType
ALU = mybir.AluOpType


DEBUG_DUMP = bool(int(__import__("os").environ.get("BASS_DBG", "0")))


def _emit(tc):
    nc = tc.nc
    xqT = nc.dram_tensor("xqT", [128, N_SB, KI, SB], BF16, kind="ExternalInput").ap()
    xkT = nc.dram_tensor("xkT", [128, N_SB, KI, SB], BF16, kind="ExternalInput").ap()
    xvT = nc.dram_tensor("xvT", [128, N_SB, KI, SB], BF16, kind="ExternalInput").ap()
    wq = nc.dram_tensor("wq", [128, KI, O], BF16, kind="ExternalInput").ap()
    wk = nc.dram_tensor("wk", [128, KI, O], BF16, kind="ExternalInput").ap()
    wv = nc.dram_tensor("wv", [128, KI, O], BF16, kind="ExternalInput").ap()
    wo = nc.dram_tensor("wo", [128, 2, H_DIM], BF16, kind="ExternalInput").ap()
    bqd = nc.dram_tensor("bq", [128, 2], FP32, kind="ExternalInput").ap()
    bkd = nc.dram_tensor("bk", [128, 2], FP32, kind="ExternalInput").ap()
    bvd = nc.dram_tensor("bv", [1, O], BF16, kind="ExternalInput").ap()
    y = nc.dram_tensor("y", [S, H_DIM], BF16, kind="ExternalOutput").ap()
    dbg = None
    if DEBUG_DUMP:
        dbg = {
            "qpt": nc.dram_tensor("d_qpt", [128, 2, S], BF16,
                                  kind="ExternalOutput").ap(),
            "kpt": nc.dram_tensor("d_kpt", [128, 2, S], BF16,
                                  kind="ExternalOutput").ap(),
            "vst": nc.dram_tensor("d_vst", [128, N_CH * HPC * 65], BF16,
                                  kind="ExternalOutput").ap(),
            "outt": nc.dram_tensor("d_outt", [128, 2, S], BF16,
                                   kind="ExternalOutput").ap(),
            "kvs": nc.dram_tensor("d_kvs", [128, N_QB, HPC, 65 * (CPB + 1)],
                                  BF16, kind="ExternalOutput").ap(),
            "sd": nc.dram_tensor("d_sd", [128, N_QB, HPC, QB], BF16,
                                 kind="ExternalOutput").ap(),
            "kn": nc.dram_tensor("d_kn", [128, N_QB, HPC, CPB * 64], BF16,
                                 kind="ExternalOutput").ap(),
        }

    with tc.tile_pool(name="singles", bufs=1) as singles:
        _emit_body(tc, singles, xqT, xkT, xvT, wq, wk, wv, wo, bqd, bkd, bvd,
                   y, dbg)


def _emit_body(tc, singles, xqT, xkT, xvT, wq, wk, wv, wo, bqd, bkd, bvd, y,
               dbg=None):
    nc = tc.nc
    # --- resident weights / constants -------------------------------------
    wq_s = singles.tile([128, KI, O], BF16, tag="wq")
    wk_s = singles.tile([128, KI, O], BF16, tag="wk")
    wv_s = singles.tile([128, KI, O], BF16, tag="wv")
    wo_s = singles.tile([128, 2, H_DIM], BF16, tag="wo")
    bq_s = singles.tile([128, 2], FP32, tag="bq")
    bk_s = singles.tile([128, 2], FP32, tag="bk")
    bv_s = singles.tile([1, O], BF16, tag="bv")

    ident = singles.tile([128, 64], BF16, tag="ident")
    make_identity(nc, ident[0:64, :])
    make_identity(nc, ident[64:128, :])
    ones = singles.tile([1, 128], BF16, tag="ones")
    nc.gpsimd.memset(ones[:], 1.0)
    umask4 = singles.tile([128, CPB * CH], BF16, tag="umask")
    make_upper_triangular(nc, umask4[:, 0:CH], val=1.0, diag=True)
    for cj in range(1, CPB):
        nc.gpsimd.tensor_copy(umask4[:, ts(cj, CH)], umask4[:, 0:CH])
    # keep mask for the KV prefix scan: 0 on seed slots, 1 elsewhere
    keep = singles.tile([64, 65, CPB + 1], BF16, tag="keep")
    nc.gpsimd.memset(keep[:], 1.0)
    nc.gpsimd.memset(keep[:, :, 0:1], 0.0)

    # --- resident activations ---------------------------------------------
    xq_s = singles.tile([128, N_SB, KI, SB], BF16, tag="xq")
    xk_s = singles.tile([128, N_SB, KI, SB], BF16, tag="xk")
    xv_s = singles.tile([128, N_SB, KI, SB], BF16, tag="xv")
    qpt = [singles.tile([128, S], BF16, tag=f"qpt{m}", name=f"qpt{m}") for m in range(2)]
    kpt = [singles.tile([128, S], BF16, tag=f"kpt{m}", name=f"kpt{m}") for m in range(2)]
    vst = singles.tile([128, N_CH, HPC, 65], BF16, tag="vst")
    nc.gpsimd.memset(vst[:, :, :, 64:65], 1.0)
    outt = [singles.tile([128, S], BF16, tag=f"outt{c}", name=f"outt{c}") for c in range(2)]
    kv0 = singles.tile([128, 65], BF16, tag="kv0")
    nc.gpsimd.memset(kv0[:], 0.0)

    # per-head running KV prefix state (AP into kvs_pool tiles after block 0)
    kv_cur = {h: kv0[ds(64 * (h % 2), 64), :] for h in range(HPC)}

    with (
        tc.tile_pool(name="phi", bufs=4) as phi_pool,
        tc.tile_pool(name="ssb", bufs=4) as ssb_pool,
        tc.tile_pool(name="kvs", bufs=2) as kvs_pool,
        tc.tile_pool(name="den", bufs=4) as den_pool,
        tc.tile_pool(name="yt", bufs=4) as yt_pool,
        tc.tile_pool(name="pqk", bufs=1, space="PSUM") as pqk_pool,
        tc.tile_pool(name="pv", bufs=1, space="PSUM") as pv_pool,
        tc.tile_pool(name="pbig", bufs=2, space="PSUM") as pbig_pool,
        tc.tile_pool(name="pnum", bufs=2, space="PSUM") as pnum_pool,
    ):

        def prefetch_x(sb, split=False):
            if split:
                nc.sync.dma_start(xq_s[:, sb, 0:4], xqT[:, sb, 0:4])
                nc.sync.dma_start(xq_s[:, sb, 4:8], xqT[:, sb, 4:8])
                nc.sync.dma_start(xv_s[:, sb, 0:4], xvT[:, sb, 0:4])
                nc.sync.dma_start(xv_s[:, sb, 4:8], xvT[:, sb, 4:8])
            else:
                nc.sync.dma_start(xq_s[:, sb], xqT[:, sb])
                nc.sync.dma_start(xv_s[:, sb], xvT[:, sb])
            nc.sync.dma_start(xk_s[:, sb], xkT[:, sb])

        def phi_evict(p_x, b_x, m, dst, scol, pref, add_eng=None):
            e_t = phi_pool.tile([128, SB], BF16, tag="e", name=f"e_{pref}")
            nc.scalar.activation(e_t[:], p_x[:], AF.Exp, bias=b_x[:, ds(m, 1)])
            r_t = phi_pool.tile([128, SB], BF16, tag="r", name=f"r_{pref}")
            nc.scalar.activation(r_t[:], p_x[:], AF.Relu, bias=b_x[:, ds(m, 1)])
            nc.vector.tensor_scalar_min(e_t[:], e_t[:], 1.0)
            (add_eng or nc.vector).tensor_add(dst[:, scol], r_t[:], e_t[:])

        def emit_A_q(sb):
            scol = ds(sb * SB, SB)
            p_q = [pqk_pool.tile([128, SB], FP32, tag=f"m{m}", name=f"pq{m}_{sb}")
                   for m in range(2)]
            for ic in range(KI):
                for m in range(2):
                    nc.tensor.matmul(
                        p_q[m][:], wq_s[:, ic, ts(m, 128)], xq_s[:, sb, ic, :],
                        start=(ic == 0), stop=(ic == KI - 1),
                    )
            for m in range(2):
                phi_evict(p_q[m], bq_s, m, qpt[m], scol, f"q{m}_{sb}")

        def emit_A_v(sb):
            p_v = [pv_pool.tile([128, SB], FP32, tag=f"v{i}", name=f"pv{i}_{sb}")
                   for i in range(2)]
            for ic in range(KI):
                for st in range(4):
                    nc.tensor.matmul(
                        p_v[st // 2][:, ds((st % 2) * O, O)],
                        xv_s[:, sb, ic, ds(st * 128, 128)], wv_s[:, ic, :],
                        start=(ic == 0 and st % 2 == 0), stop=False,
                    )
            for st in range(4):
                nc.tensor.matmul(
                    p_v[st // 2][:, ds((st % 2) * O, O)], ones[:, 0:128], bv_s[:],
                    start=False, stop=(st % 2 == 1),
                )
            for st in range(4):
                c = sb * 4 + st
                nc.scalar.copy(
                    vst[:, c, :, 0:64],
                    p_v[st // 2][:, ds((st % 2) * O, O)].rearrange(
                        "p (h d) -> p h d", h=HPC),
                )

        def emit_A_k(sb):
            scol = ds(sb * SB, SB)
            p_k = [pqk_pool.tile([128, SB], FP32, tag=f"m{m}", name=f"pk{m}_{sb}")
                   for m in range(2)]
            for ic in range(KI):
                for m in range(2):
                    nc.tensor.matmul(
                        p_k[m][:], wk_s[:, ic, ts(m, 128)], xk_s[:, sb, ic, :],
                        start=(ic == 0), stop=(ic == KI - 1),
                    )
            for m in range(2):
                phi_evict(p_k[m], bk_s, m, kpt[m], scol, f"k{m}_{sb}", add_eng=nc.vector)

        pending_muls = []

        def flush_muls():
            for f in pending_muls:
                nc.vector.tensor_mul(*f())
            pending_muls.clear()

        def attn_front(qb, heads):
            """Scores/masks/transposes/num-diag/kv-outers/scan for a pair."""
            ctx = {}
            for h in heads:
                mt, prow = h // 2, 64 * (h % 2)
                ctx[h] = dict(
                    mt=mt, prow=prow,
                    qp=qpt[mt][ds(prow, 64), :],
                    kp=kpt[mt][ds(prow, 64), :],
                )
            # diagonal scores (masked) + transposes
            for h in heads:
                x = ctx[h]
                p_sd = pbig_pool.tile([128, QB], FP32, tag="big",
                                      name=f"psd{qb}_{h}")
                for cj in range(CPB):
                    c = qb * CPB + cj
                    nc.tensor.matmul(
                        p_sd[:, ts(cj, CH)], x["kp"][:, ds(c * CH, CH)],
                        x["qp"][:, ds(qb * QB + cj * CH, CH)],
                        start=(cj == 0), stop=(cj == CPB - 1),
                    )
                x["p_sd"] = p_sd
            for h in heads:
                x = ctx[h]
                sd_t = ssb_pool.tile([128, QB], BF16, tag="sd",
                                     name=f"sd{qb}_{h}")
                nc.vector.tensor_mul(sd_t[:], x["p_sd"][:], umask4[:])
                x["sd_t"] = sd_t
            # flush the deferred out-muls of pair k-2 BEFORE allocating
            # p_num (their read of the recycled pnum buffer must precede
            # the next alloc in emission order), but AFTER this pair's
            # masks so the DVE queue has work while the broadcast lands.
            flush_muls()
            for h in heads:
                x = ctx[h]
                x["p_num"] = pnum_pool.tile([65, QB], FP32, tag="num",
                                            name=f"num{qb}_{h}")
            for h in heads:
                x = ctx[h]
                p_kn = pbig_pool.tile([128, CPB, 64], BF16, tag="big",
                                      name=f"pkn{qb}_{h}")
                for cj in range(CPB):
                    c = qb * CPB + cj
                    nc.tensor.matmul(
                        p_kn[:, cj, :], x["kp"][:, ds(c * CH, CH)],
                        ident[ds(x["prow"], 64), :], is_transpose=True,
                        start=(cj == 0), stop=(cj == CPB - 1),
                    )
                x["p_kn"] = p_kn
            for h in heads:
                x = ctx[h]
                kn_t = ssb_pool.tile([128, CPB, 64], BF16, tag="kn",
                                     name=f"kn{qb}_{h}")
                nc.scalar.copy(kn_t[:], x["p_kn"][:])
                x["kn_t"] = kn_t
            # num: diagonal contributions
            for h in heads:
                x = ctx[h]
                for cj in range(CPB):
                    c = qb * CPB + cj
                    nc.tensor.matmul(
                        x["p_num"][:, ts(cj, CH)], vst[:, c, h, :],
                        x["sd_t"][:, ts(cj, CH)],
                        start=(cj == 0), stop=False,
                    )
            # KV prefix tile: slot 0 seeded with kv state via identity MM,
            # slots 1..4 = per-chunk outer products (independent groups)
            for h in heads:
                x = ctx[h]
                p_kv5 = pbig_pool.tile([64, 65, CPB + 1], FP32, tag="big",
                                       name=f"pkv{qb}_{h}")
                nc.tensor.matmul(
                    p_kv5[:, :, 0], ident[ds(x["prow"], 64), :], kv_cur[h],
                    start=True, stop=False,
                )
                for cj in range(CPB):
                    c = qb * CPB + cj
                    nc.tensor.matmul(
                        p_kv5[:, :, cj + 1], x["kn_t"][:, cj, :],
                        vst[:, c, h, :], start=False, stop=(cj == CPB - 1),
                    )
                x["p_kv5"] = p_kv5
            # one scan per head: state = keep*state + p_kv5 (segmented per
            # v-column) -> prefix states land in SBUF at prow
            for h in heads:
                x = ctx[h]
                kvs_t = kvs_pool.tile([128, 65, CPB + 1], BF16, tag=f"kvp{h}",
                                      name=f"kvp{qb}_{h}")
                kvs = kvs_t[ds(x["prow"], 64), :, :]
                nc.vector.tensor_tensor_scan(
                    kvs.rearrange("p a b -> p (a b)"),
                    keep[:].rearrange("p a b -> p (a b)"),
                    x["p_kv5"][:].rearrange("p a b -> p (a b)"),
                    0.0, ALU.mult, ALU.add,
                )
                x["kvs"] = kvs
                kv_cur[h] = kvs[:, :, CPB]
                if dbg is not None:
                    nc.sync.dma_start(
                        dbg["kvs"][:, qb, h, :],
                        kvs_t[:].rearrange("p a b -> p (a b)"))
                    nc.sync.dma_start(
                        dbg["sd"][:, qb, h, :], x["sd_t"][:])
                    nc.sync.dma_start(
                        dbg["kn"][:, qb, h, :],
                        x["kn_t"][:].rearrange("p a b -> p (a b)"))
            ctx["_qb"], ctx["_heads"] = qb, heads
            return ctx

        def attn_back_pe(ctx):
            """num-inter + den chain for a pair whose front is emitted.
            The out-mul is deferred to the NEXT back_pe (DVE queue never
            waits on the broadcast DMA)."""
            qb, heads = ctx["_qb"], ctx["_heads"]
            qcol = ds(qb * QB, QB)
            # num: off-diagonal/inter contributions via prefix states
            for h in heads:
                x = ctx[h]
                for cj in range(CPB):
                    if qb == 0 and cj == 0:
                        continue
                    nc.tensor.matmul(
                        x["p_num"][:, ts(cj, CH)], x["kvs"][:, :, cj],
                        x["qp"][:, ds(qb * QB + cj * CH, CH)],
                        start=False, stop=(cj == CPB - 1),
                    )
            flush_muls()
            # den: reciprocal straight off the PSUM num row, broadcast to
            # 64 partitions on gpsimd (only op gpsimd runs in steady state,
            # so its library never reloads). The deferred mul absorbs the
            # broadcast latency.
            for h in heads:
                x = ctx[h]
                # ACT copies the den row to partition 0 (engines other than
                # ACT mishandle the cross-partition move), then DVE recip +
                # gpsimd broadcast, all exactly as the proven baseline.
                den_t = den_pool.tile([1, QB], FP32, tag="den",
                                      name=f"den{qb}_{h}")
                nc.scalar.copy(den_t[:], x["p_num"][ds(64, 1), :])
                rden = den_pool.tile([1, QB], FP32, tag="rden",
                                     name=f"rden{qb}_{h}")
                nc.vector.reciprocal_approx_fast(rden[:], den_t[:])
                bc = den_pool.tile([64, QB], FP32, tag="bc",
                                   name=f"bc{qb}_{h}")
                nc.gpsimd.partition_broadcast(bc[:], rden[:])

                def mul(x=x, bc=bc, qcol=qcol):
                    return (outt[x["mt"]][ds(x["prow"], 64), qcol],
                            x["p_num"][0:64, :], bc[:])
                pending_muls.append(mul)

        def emit_C(sts):
            nst = len(sts)
            st0 = sts[0]
            y_t = yt_pool.tile([128, nst, H_DIM], BF16, tag="y",
                               name=f"yt{st0}")
            for i, st in enumerate(sts):
                for n in range(2):
                    # O-proj PSUM shares the pqk tag rings (PSUM is full)
                    p_o = pqk_pool.tile([128, 512], FP32, tag=f"m{n}",
                                        name=f"po{st}_{n}")
                    for ct in range(2):
                        nc.tensor.matmul(
                            p_o[:], outt[ct][:, ts(st, 128)],
                            wo_s[:, ct, ts(n, 512)],
                            start=(ct == 0), stop=(ct == 1),
                        )
                    if n == 0:
                        nc.scalar.copy(y_t[:, i, ts(n, 512)], p_o[:])
                    else:
                        nc.vector.tensor_copy(y_t[:, i, ts(n, 512)], p_o[:])
            nc.sync.dma_start(
                y[ds(st0 * 128, nst * 128), :].rearrange(
                    "(i p) c -> p i c", p=128),
                y_t[:],
            )

        # ---- emission schedule -------------------------------------------
        # DMA order: first A's operands first, then stay a stage ahead.
        nc.sync.dma_start(wq_s[:, 0:4, :], wq[:, 0:4, :])
        nc.sync.dma_start(xq_s[:, 0, 0:4], xqT[:, 0, 0:4])
        nc.sync.dma_start(wq_s[:, 4:8, :], wq[:, 4:8, :])
        nc.sync.dma_start(xq_s[:, 0, 4:8], xqT[:, 0, 4:8])
        nc.sync.dma_start(bq_s[:], bqd)
        nc.sync.dma_start(wv_s[:], wv)
        nc.sync.dma_start(xv_s[:, 0, 0:4], xvT[:, 0, 0:4])
        nc.sync.dma_start(xv_s[:, 0, 4:8], xvT[:, 0, 4:8])
        nc.sync.dma_start(bv_s[:], bvd)
        nc.sync.dma_start(wk_s[:], wk)
        nc.sync.dma_start(xk_s[:, 0, 0:4], xkT[:, 0, 0:4])
        nc.sync.dma_start(xk_s[:, 0, 4:8], xkT[:, 0, 4:8])
        nc.sync.dma_start(bk_s[:], bkd)
        emit_A_q(0)
        prefetch_x(1)
        nc.sync.dma_start(wo_s[:], wo)
        emit_A_v(0)
        emit_A_k(0)
        emit_A_v(1)
        emit_A_q(1)
        f0 = attn_front(0, [0, 1])
        emit_A_k(1)
        f1 = attn_front(0, [2, 3])
        attn_back_pe(f0)
        prefetch_x(2)
        emit_A_q(2)
        f2 = attn_front(1, [0, 1])
        attn_back_pe(f1)
        emit_A_v(2)
        f3 = attn_front(1, [2, 3])
        attn_back_pe(f2)
        emit_C([0, 1])
        emit_C([2, 3])
        emit_A_k(2)
        prefetch_x(3)
        emit_A_q(3)
        f4 = attn_front(2, [0, 1])
        attn_back_pe(f3)
        emit_A_v(3)
        f5 = attn_front(2, [2, 3])
        attn_back_pe(f4)
        emit_C([4, 5])
        emit_C([6, 7])
        emit_A_k(3)
        f6 = attn_front(3, [0, 1])
        attn_back_pe(f5)
        f7 = attn_front(3, [2, 3])
        emit_C([8, 9])
        attn_back_pe(f6)
        emit_C([10, 11])
        attn_back_pe(f7)
        flush_muls()
        emit_C([12, 13])
        emit_C([14, 15])
        if dbg is not None:
            for m in range(2):
                nc.sync.dma_start(dbg["qpt"][:, m, :], qpt[m][:])
                nc.sync.dma_start(dbg["kpt"][:, m, :], kpt[m][:])
                nc.sync.dma_start(dbg["outt"][:, m, :], outt[m][:])
            nc.sync.dma_start(
                dbg["vst"][:],
                vst[:].rearrange("p a b c -> p (a b c)"))


_PROGRAM = None


def _get_program():
    global _PROGRAM
    if _PROGRAM is None:
        nc = bacc.Bacc("TRN2", target_bir_lowering=False, debug=False)
        with tile.TileContext(nc) as tc:
            _emit(tc)
        nc.compile()
        _PROGRAM = nc
    return _PROGRAM


def make_in_maps(inputs):
    BF = ml_dtypes.bfloat16
    query, key, value = (np.asarray(inputs[k], np.float32)
                         for k in ("query", "key", "value"))
    Wq, Wk, Wv, Wo = (np.asarray(inputs[k], np.float32)
                      for k in ("Wq", "Wk", "Wv", "Wo"))
    bq, bk, bv = (np.asarray(inputs[k], np.float32) for k in ("bq", "bk", "bv"))

    def xprep(x, b):  # (2048, 1024) -> (128, N_SB, KI, SB) bf16
        t = x[b].T.reshape(KI, 128, N_SB, SB)
        return np.ascontiguousarray(t.transpose(1, 2, 0, 3)).astype(BF)

    def wslice(W, g):  # (1024, 256)-slice -> (128, 8, 256) bf16
        wt = W[g * O:(g + 1) * O].T  # (1024, 256)
        return np.ascontiguousarray(
            wt.reshape(KI, 128, O).transpose(1, 0, 2)
        ).astype(BF)

    in_maps = []
    for c in range(N_CORES):
        b, g = divmod(c, 4)
        sl = slice(g * O, (g + 1) * O)
        in_maps.append({
            "xqT": xprep(query, b),
            "xkT": xprep(key, b),
            "xvT": xprep(value, b),
            "wq": wslice(Wq, g),
            "wk": wslice(Wk, g),
            "wv": wslice(Wv, g),
            "wo": np.ascontiguousarray(
                Wo[:, sl].T.reshape(2, 128, H_DIM).transpose(1, 0, 2)
            ).astype(BF),
            "bq": np.ascontiguousarray(bq[sl].reshape(2, 128).T),
            "bk": np.ascontiguousarray(bk[sl].reshape(2, 128).T),
            "bv": np.ascontiguousarray(bv[sl].reshape(1, O)).astype(BF),
        })
    return in_maps


def kernel(query, key, value, Wq, bq, Wk, bk, Wv, bv, Wo, bo, _trace=False):
    bo = np.asarray(bo, np.float32)
    in_maps = make_in_maps(dict(
        query=query, key=key, value=value, Wq=Wq, Wk=Wk, Wv=Wv, Wo=Wo,
        bq=bq, bk=bk, bv=bv,
    ))

    nc = _get_program()
    res = run_bass_kernel_spmd(
        nc, in_maps, core_ids=list(range(N_CORES)), trace=_trace
    )
    out = np.empty((B, S, H_DIM), np.float32)
    for b in range(B):
        acc = res.results[4 * b]["y"].astype(np.float32)
        for g in range(1, 4):
            acc += res.results[4 * b + g]["y"].astype(np.float32)
        out[b] = acc + bo
    if _trace:
        kernel.last_result = res
    return out


# revision 23
# speedup vs baseline: 1.2614x; 1.0040x over previous
"""Causal linear attention (ELU+1 feature map) for Trainium2, 8 NeuronCores.

Sharding: core c handles batch b = c // 4 and head-group g = c % 4
(4 heads of 64 dims -> a 256-feature slice of the QKV/O projections).
Each core computes its partial O-projection output (2048, 1024) in bf16;
the host sums the 4 partials per batch in fp32 and adds bo.

v4: software-pipelined attention. Per 512-query block: masked diagonal
128-chunk scores (PE + DVE mask), off-diagonal via per-chunk KV prefix
states built with one DVE tensor_tensor_scan per (block, head). front()
emits scores/masks/transposes/num-diag/kv-outers/scan; back_pe() emits
the num-inter matmuls one pair later so the PE queue never waits on the
scan. The den reciprocal reads the PSUM num row directly (DVE), is
broadcast to 64 partitions by a sync-queue SBUF->SBUF DMA (partition
stride 0), and the out = num * rden mul is deferred one more pair so the
DVE queue never waits on the broadcast DMA. gpsimd only does startup
memsets and the q-path phi adds. O-projection + y DMA are spread across
blocks (emit_C right after each block's attention) instead of all at the
tail; y DMAs fire from the sync queue.

x DRAM layout is [128, N_SB, KI, SB] so each 512-column block transfer
is one contiguous 8KB run per partition.
"""

import numpy as np
import ml_dtypes

import concourse.bacc as bacc
import concourse.bass as bass
import concourse.mybir as mybir
import concourse.tile as tile
from concourse.bass import ds, ts
from concourse.bass_utils import run_bass_kernel_spmd
from concourse.masks import make_identity, make_upper_triangular

B, S, H_DIM = 2, 2048, 1024
N_HEADS, HEAD_DIM = 16, 64
EPS = 1e-6

N_CORES = 8
HPC = 4                  # heads per core
O = HPC * HEAD_DIM       # 256: per-core projection feature slice
CH = 128                 # key chunk
QB = 512                 # query block
N_CH = S // CH           # 16
N_QB = S // QB           # 4
CPB = QB // CH           # 4 chunks per query block
KI = H_DIM // 128        # 8 contraction chunks
SB = 512                 # projection s-block width
N_SB = S // SB           # 4

FP32 = mybir.dt.float32
BF16 = mybir.dt.bfloat16

AF = mybir.ActivationFunctionType
ALU = mybir.AluOpType


DEBUG_DUMP = bool(int(__import__("os").environ.get("BASS_DBG", "0")))


def _emit(tc):
    nc = tc.nc
    xqT = nc.dram_tensor("xqT", [128, N_SB, KI, SB], BF16, kind="ExternalInput").ap()
    xkT = nc.dram_tensor("xkT", [128, N_SB, KI, SB], BF16, kind="ExternalInput").ap()
    xvT = nc.dram_tensor("xvT", [128, N_SB, KI, SB], BF16, kind="ExternalInput").ap()
    wq = nc.dram_tensor("wq", [128, KI, O], BF16, kind="ExternalInput").ap()
    wk = nc.dram_tensor("wk", [128, KI, O], BF16, kind="ExternalInput").ap()
    wv = nc.dram_tensor("wv", [128, KI, O], BF16, kind="ExternalInput").ap()
    wo = nc.dram_tensor("wo", [128, 2, H_DIM], BF16, kind="ExternalInput").ap()
    bqd = nc.dram_tensor("bq", [128, 2], FP32, kind="ExternalInput").ap()
    bkd = nc.dram_tensor("bk", [128, 2], FP32, kind="ExternalInput").ap()
    bvd = nc.dram_tensor("bv", [1, O], BF16, kind="ExternalInput").ap()
    y = nc.dram_tensor("y", [S, H_DIM], BF16, kind="ExternalOutput").ap()
    dbg = None
    if DEBUG_DUMP:
        dbg = {
            "qpt": nc.dram_tensor("d_qpt", [128, 2, S], BF16,
                                  kind="ExternalOutput").ap(),
            "kpt": nc.dram_tensor("d_kpt", [128, 2, S], BF16,
                                  kind="ExternalOutput").ap(),
            "vst": nc.dram_tensor("d_vst", [128, N_CH * HPC * 65], BF16,
                                  kind="ExternalOutput").ap(),
            "outt": nc.dram_tensor("d_outt", [128, 2, S], BF16,
                                   kind="ExternalOutput").ap(),
            "kvs": nc.dram_tensor("d_kvs", [128, N_QB, HPC, 65 * (CPB + 1)],
                                  BF16, kind="ExternalOutput").ap(),
            "sd": nc.dram_tensor("d_sd", [128, N_QB, HPC, QB], BF16,
                                 kind="ExternalOutput").ap(),
            "kn": nc.dram_tensor("d_kn", [128, N_QB, HPC, CPB * 64], BF16,
                                 kind="ExternalOutput").ap(),
        }

    with tc.tile_pool(name="singles", bufs=1) as singles:
        _emit_body(tc, singles, xqT, xkT, xvT, wq, wk, wv, wo, bqd, bkd, bvd,
                   y, dbg)


def _emit_body(tc, singles, xqT, xkT, xvT, wq, wk, wv, wo, bqd, bkd, bvd, y,
               dbg=None):
    nc = tc.nc
    # --- resident weights / constants -------------------------------------
    wq_s = singles.tile([128, KI, O], BF16, tag="wq")
    wk_s = singles.tile([128, KI, O], BF16, tag="wk")
    wv_s = singles.tile([128, KI, O], BF16, tag="wv")
    wo_s = singles.tile([128, 2, H_DIM], BF16, tag="wo")
    bq_s = singles.tile([128, 2], FP32, tag="bq")
    bk_s = singles.tile([128, 2], FP32, tag="bk")
    bv_s = singles.tile([1, O], BF16, tag="bv")

    ident = singles.tile([128, 64], BF16, tag="ident")
    make_identity(nc, ident[0:64, :])
    make_identity(nc, ident[64:128, :])
    ones = singles.tile([1, 128], BF16, tag="ones")
    nc.gpsimd.memset(ones[:], 1.0)
    umask4 = singles.tile([128, CPB * CH], BF16, tag="umask")
    make_upper_triangular(nc, umask4[:, 0:CH], val=1.0, diag=True)
    for cj in range(1, CPB):
        nc.gpsimd.tensor_copy(umask4[:, ts(cj, CH)], umask4[:, 0:CH])
    # keep mask for the KV prefix scan: 0 on seed slots, 1 elsewhere
    keep = singles.tile([64, 65, CPB + 1], BF16, tag="keep")
    nc.gpsimd.memset(keep[:], 1.0)
    nc.gpsimd.memset(keep[:, :, 0:1], 0.0)

    # --- resident activations ---------------------------------------------
    xq_s = singles.tile([128, N_SB, KI, SB], BF16, tag="xq")
    xk_s = singles.tile([128, N_SB, KI, SB], BF16, tag="xk")
    xv_s = singles.tile([128, N_SB, KI, SB], BF16, tag="xv")
    qpt = [singles.tile([128, S], BF16, tag=f"qpt{m}", name=f"qpt{m}") for m in range(2)]
    kpt = [singles.tile([128, S], BF16, tag=f"kpt{m}", name=f"kpt{m}") for m in range(2)]
    vst = singles.tile([128, N_CH, HPC, 65], BF16, tag="vst")
    nc.gpsimd.memset(vst[:, :, :, 64:65], 1.0)
    outt = [singles.tile([128, S], BF16, tag=f"outt{c}", name=f"outt{c}") for c in range(2)]
    kv0 = singles.tile([128, 65], BF16, tag="kv0")
    nc.gpsimd.memset(kv0[:], 0.0)

    # per-head running KV prefix state (AP into kvs_pool tiles after block 0)
    kv_cur = {h: kv0[ds(64 * (h % 2), 64), :] for h in range(HPC)}

    with (
        tc.tile_pool(name="phi", bufs=4) as phi_pool,
        tc.tile_pool(name="ssb", bufs=4) as ssb_pool,
        tc.tile_pool(name="kvs", bufs=2) as kvs_pool,
        tc.tile_pool(name="den", bufs=4) as den_pool,
        tc.tile_pool(name="yt", bufs=4) as yt_pool,
        tc.tile_pool(name="pqk", bufs=1, space="PSUM") as pqk_pool,
        tc.tile_pool(name="pv", bufs=1, space="PSUM") as pv_pool,
        tc.tile_pool(name="pbig", bufs=2, space="PSUM") as pbig_pool,
        tc.tile_pool(name="pnum", bufs=2, space="PSUM") as pnum_pool,
    ):

        def prefetch_x(sb, split=False):
            if split:
                nc.sync.dma_start(xq_s[:, sb, 0:4], xqT[:, sb, 0:4])
                nc.sync.dma_start(xq_s[:, sb, 4:8], xqT[:, sb, 4:8])
                nc.sync.dma_start(xv_s[:, sb, 0:4], xvT[:, sb, 0:4])
                nc.sync.dma_start(xv_s[:, sb, 4:8], xvT[:, sb, 4:8])
            else:
                nc.sync.dma_start(xq_s[:, sb], xqT[:, sb])
                nc.sync.dma_start(xv_s[:, sb], xvT[:, sb])
            nc.sync.dma_start(xk_s[:, sb], xkT[:, sb])

        def phi_evict(p_x, b_x, m, dst, scol, pref, add_eng=None):
            e_t = phi_pool.tile([128, SB], BF16, tag="e", name=f"e_{pref}")
            nc.scalar.activation(e_t[:], p_x[:], AF.Exp, bias=b_x[:, ds(m, 1)])
            r_t = phi_pool.tile([128, SB], BF16, tag="r", name=f"r_{pref}")
            nc.scalar.activation(r_t[:], p_x[:], AF.Relu, bias=b_x[:, ds(m, 1)])
            nc.vector.tensor_scalar_min(e_t[:], e_t[:], 1.0)
            (add_eng or nc.vector).tensor_add(dst[:, scol], r_t[:], e_t[:])

        def emit_A_q(sb):
            scol = ds(sb * SB, SB)
            p_q = [pqk_pool.tile([128, SB], FP32, tag=f"m{m}", name=f"pq{m}_{sb}")
                   for m in range(2)]
            for ic in range(KI):
                for m in range(2):
                    nc.tensor.matmul(
                        p_q[m][:], wq_s[:, ic, ts(m, 128)], xq_s[:, sb, ic, :],
                        start=(ic == 0), stop=(ic == KI - 1),
                    )
            for m in range(2):
                phi_evict(p_q[m], bq_s, m, qpt[m], scol, f"q{m}_{sb}")

        def emit_A_v(sb):
            p_v = [pv_pool.tile([128, SB], FP32, tag=f"v{i}", name=f"pv{i}_{sb}")
                   for i in range(2)]
            for ic in range(KI):
                for st in range(4):
                    nc.tensor.matmul(
                        p_v[st // 2][:, ds((st % 2) * O, O)],
                        xv_s[:, sb, ic, ds(st * 128, 128)], wv_s[:, ic, :],
                        start=(ic == 0 and st % 2 == 0), stop=False,
                    )
            for st in range(4):
                nc.tensor.matmul(
                    p_v[st // 2][:, ds((st % 2) * O, O)], ones[:, 0:128], bv_s[:],
                    start=False, stop=(st % 2 == 1),
                )
            for st in range(4):
                c = sb * 4 + st
                nc.scalar.copy(
                    vst[:, c, :, 0:64],
                    p_v[st // 2][:, ds((st % 2) * O, O)].rearrange(
                        "p (h d) -> p h d", h=HPC),
                )

        def emit_A_k(sb):
            scol = ds(sb * SB, SB)
            p_k = [pqk_pool.tile([128, SB], FP32, tag=f"m{m}", name=f"pk{m}_{sb}")
                   for m in range(2)]
            for ic in range(KI):
                for m in range(2):
                    nc.tensor.matmul(
                        p_k[m][:], wk_s[:, ic, ts(m, 128)], xk_s[:, sb, ic, :],
                        start=(ic == 0), stop=(ic == KI - 1),
                    )
            for m in range(2):
                phi_evict(p_k[m], bk_s, m, kpt[m], scol, f"k{m}_{sb}", add_eng=nc.vector)

        pending_muls = []

        def flush_muls():
            for f in pending_muls:
                nc.vector.tensor_mul(*f())
            pending_muls.clear()

        def attn_front(qb, heads):
            """Scores/masks/transposes/num-diag/kv-outers/scan for a pair."""
            ctx = {}
            for h in heads:
                mt, prow = h // 2, 64 * (h % 2)
                ctx[h] = dict(
                    mt=mt, prow=prow,
                    qp=qpt[mt][ds(prow, 64), :],
                    kp=kpt[mt][ds(prow, 64), :],
                )
            # diagonal scores (masked) + transposes
            for h in heads:
                x = ctx[h]
                p_sd = pbig_pool.tile([128, QB], FP32, tag="big",
                                      name=f"psd{qb}_{h}")
                for cj in range(CPB):
                    c = qb * CPB + cj
                    nc.tensor.matmul(
                        p_sd[:, ts(cj, CH)], x["kp"][:, ds(c * CH, CH)],
                        x["qp"][:, ds(qb * QB + cj * CH, CH)],
                        start=(cj == 0), stop=(cj == CPB - 1),
                    )
                x["p_sd"] = p_sd
            for h in heads:
                x = ctx[h]
                sd_t = ssb_pool.tile([128, QB], BF16, tag="sd",
                                     name=f"sd{qb}_{h}")
                nc.vector.tensor_mul(sd_t[:], x["p_sd"][:], umask4[:])
                x["sd_t"] = sd_t
            # flush the deferred out-muls of pair k-2 BEFORE allocating
            # p_num (their read of the recycled pnum buffer must precede
            # the next alloc in emission order), but AFTER this pair's
            # masks so the DVE queue has work while the broadcast lands.
            flush_muls()
            for h in heads:
                x = ctx[h]
                x["p_num"] = pnum_pool.tile([65, QB], FP32, tag="num",
                                            name=f"num{qb}_{h}")
            for h in heads:
                x = ctx[h]
                p_kn = pbig_pool.tile([128, CPB, 64], BF16, tag="big",
                                      name=f"pkn{qb}_{h}")
                for cj in range(CPB):
                    c = qb * CPB + cj
                    nc.tensor.matmul(
                        p_kn[:, cj, :], x["kp"][:, ds(c * CH, CH)],
                        ident[ds(x["prow"], 64), :], is_transpose=True,
                        start=(cj == 0), stop=(cj == CPB - 1),
                    )
                x["p_kn"] = p_kn
            for h in heads:
                x = ctx[h]
                kn_t = ssb_pool.tile([128, CPB, 64], BF16, tag="kn",
                                     name=f"kn{qb}_{h}")
                nc.scalar.copy(kn_t[:], x["p_kn"][:])
                x["kn_t"] = kn_t
            # num: diagonal contributions
            for h in heads:
                x = ctx[h]
                for cj in range(CPB):
                    c = qb * CPB + cj
                    nc.tensor.matmul(
                        x["p_num"][:, ts(cj, CH)], vst[:, c, h, :],
                        x["sd_t"][:, ts(cj, CH)],
                        start=(cj == 0), stop=False,
                    )
            # KV prefix tile: slot 0 seeded with kv state via identity MM,
            # slots 1..4 = per-chunk outer products (independent groups)
            for h in heads:
                x = ctx[h]
                p_kv5 = pbig_pool.tile([64, 65, CPB + 1], FP32, tag="big",
                                       name=f"pkv{qb}_{h}")
                nc.tensor.matmul(
                    p_kv5[:, :, 0], ident[ds(x["prow"], 64), :], kv_cur[h],
                    start=True, stop=False,
                )
                for cj in range(CPB):
                    c = qb * CPB + cj
                    nc.tensor.matmul(
                        p_kv5[:, :, cj + 1], x["kn_t"][:, cj, :],
                        vst[:, c, h, :], start=False, stop=(cj == CPB - 1),
                    )
                x["p_kv5"] = p_kv5
            # one scan per head: state = keep*state + p_kv5 (segmented per
            # v-column) -> prefix states land in SBUF at prow
            for h in heads:
                x = ctx[h]
                kvs_t = kvs_pool.tile([128, 65, CPB + 1], BF16, tag=f"kvp{h}",
                                      name=f"kvp{qb}_{h}")
                kvs = kvs_t[ds(x["prow"], 64), :, :]
                nc.vector.tensor_tensor_scan(
                    kvs.rearrange("p a b -> p (a b)"),
                    keep[:].rearrange("p a b -> p (a b)"),
                    x["p_kv5"][:].rearrange("p a b -> p (a b)"),
                    0.0, ALU.mult, ALU.add,
                )
                x["kvs"] = kvs
                kv_cur[h] = kvs[:, :, CPB]
                if dbg is not None:
                    nc.sync.dma_start(
                        dbg["kvs"][:, qb, h, :],
                        kvs_t[:].rearrange("p a b -> p (a b)"))
                    nc.sync.dma_start(
                        dbg["sd"][:, qb, h, :], x["sd_t"][:])
                    nc.sync.dma_start(
                        dbg["kn"][:, qb, h, :],
                        x["kn_t"][:].rearrange("p a b -> p (a b)"))
            ctx["_qb"], ctx["_heads"] = qb, heads
            return ctx

        def attn_back_pe(ctx):
            """num-inter + den chain for a pair whose front is emitted.
            The out-mul is deferred to the NEXT back_pe (DVE queue never
            waits on the broadcast DMA)."""
            qb, heads = ctx["_qb"], ctx["_heads"]
            qcol = ds(qb * QB, QB)
            # num: off-diagonal/inter contributions via prefix states
            for h in heads:
                x = ctx[h]
                for cj in range(CPB):
                    if qb == 0 and cj == 0:
                        continue
                    nc.tensor.matmul(
                        x["p_num"][:, ts(cj, CH)], x["kvs"][:, :, cj],
                        x["qp"][:, ds(qb * QB + cj * CH, CH)],
                        start=False, stop=(cj == CPB - 1),
                    )
            flush_muls()
            # den: reciprocal straight off the PSUM num row, broadcast to
            # 64 partitions on gpsimd (only op gpsimd runs in steady state,
            # so its library never reloads). The deferred mul absorbs the
            # broadcast latency.
            for h in heads:
                x = ctx[h]
                # ACT copies the den row to partition 0 (engines other than
                # ACT mishandle the cross-partition move), then DVE recip +
                # gpsimd broadcast, all exactly as the proven baseline.
                den_t = den_pool.tile([1, QB], FP32, tag="den",
                                      name=f"den{qb}_{h}")
                nc.scalar.copy(den_t[:], x["p_num"][ds(64, 1), :])
                rden = den_pool.tile([1, QB], FP32, tag="rden",
                                     name=f"rden{qb}_{h}")
                nc.vector.reciprocal_approx_fast(rden[:], den_t[:])
                bc = den_pool.tile([64, QB], FP32, tag="bc",
                                   name=f"bc{qb}_{h}")
                nc.gpsimd.partition_broadcast(bc[:], rden[:])

                def mul(x=x, bc=bc, qcol=qcol):
                    return (outt[x["mt"]][ds(x["prow"], 64), qcol],
                            x["p_num"][0:64, :], bc[:])
                pending_muls.append(mul)

        def emit_C(sts):
            nst = len(sts)
            st0 = sts[0]
            y_t = yt_pool.tile([128, nst, H_DIM], BF16, tag="y",
                               name=f"yt{st0}")
            for i, st in enumerate(sts):
                for n in range(2):
                    # O-proj PSUM shares the pqk tag rings (PSUM is full)
                    p_o = pqk_pool.tile([128, 512], FP32, tag=f"m{n}",
                                        name=f"po{st}_{n}")
                    for ct in range(2):
                        nc.tensor.matmul(
                            p_o[:], outt[ct][:, ts(st, 128)],
                            wo_s[:, ct, ts(n, 512)],
                            start=(ct == 0), stop=(ct == 1),
                        )
                    # ACT for both halves: keeps DVE free for the scans
                    nc.scalar.copy(y_t[:, i, ts(n, 512)], p_o[:])
            nc.sync.dma_start(
                y[ds(st0 * 128, nst * 128), :].rearrange(
                    "(i p) c -> p i c", p=128),
                y_t[:],
            )

        # ---- emission schedule -------------------------------------------
        # DMA order: first A's operands first (finely sliced so the first
        # matmul starts ASAP), then stay a stage ahead.
        nc.sync.dma_start(wq_s[:, 0:2, :], wq[:, 0:2, :])
        nc.sync.dma_start(xq_s[:, 0, 0:2], xqT[:, 0, 0:2])
        nc.sync.dma_start(bq_s[:], bqd)
        nc.sync.dma_start(wq_s[:, 2:8, :], wq[:, 2:8, :])
        nc.sync.dma_start(xq_s[:, 0, 2:4], xqT[:, 0, 2:4])
        nc.sync.dma_start(xq_s[:, 0, 4:8], xqT[:, 0, 4:8])
        nc.sync.dma_start(wv_s[:], wv)
        nc.sync.dma_start(xv_s[:, 0, 0:4], xvT[:, 0, 0:4])
        nc.sync.dma_start(xv_s[:, 0, 4:8], xvT[:, 0, 4:8])
        nc.sync.dma_start(bv_s[:], bvd)
        nc.sync.dma_start(wk_s[:], wk)
        nc.sync.dma_start(xk_s[:, 0, 0:4], xkT[:, 0, 0:4])
        nc.sync.dma_start(xk_s[:, 0, 4:8], xkT[:, 0, 4:8])
        nc.sync.dma_start(bk_s[:], bkd)
        emit_A_q(0)
        # block-1 x in consumption order: A_v(1) runs before A_q(1)
        nc.sync.dma_start(xv_s[:, 1], xvT[:, 1])
        nc.sync.dma_start(xq_s[:, 1], xqT[:, 1])
        nc.sync.dma_start(xk_s[:, 1], xkT[:, 1])
        emit_A_v(0)
        emit_A_k(0)
        emit_A_v(1)
        emit_A_q(1)
        f0 = attn_front(0, [0, 1])
        emit_A_k(1)
        f1 = attn_front(0, [2, 3])
        attn_back_pe(f0)
        prefetch_x(2)
        nc.sync.dma_start(wo_s[:], wo)
        emit_A_q(2)
        f2 = attn_front(1, [0, 1])
        attn_back_pe(f1)
        emit_A_v(2)
        f3 = attn_front(1, [2, 3])
        attn_back_pe(f2)
        emit_C([0, 1])
        emit_C([2, 3])
        emit_A_k(2)
        prefetch_x(3)
        emit_A_q(3)
        f4 = attn_front(2, [0, 1])
        attn_back_pe(f3)
        emit_A_v(3)
        f5 = attn_front(2, [2, 3])
        attn_back_pe(f4)
        emit_C([4, 5])
        emit_C([6, 7])
        emit_A_k(3)
        f6 = attn_front(3, [0, 1])
        attn_back_pe(f5)
        f7 = attn_front(3, [2, 3])
        emit_C([8, 9])
        attn_back_pe(f6)
        emit_C([10, 11])
        attn_back_pe(f7)
        flush_muls()
        emit_C([12, 13])
        emit_C([14, 15])
        if dbg is not None:
            for m in range(2):
                nc.sync.dma_start(dbg["qpt"][:, m, :], qpt[m][:])
                nc.sync.dma_start(dbg["kpt"][:, m, :], kpt[m][:])
                nc.sync.dma_start(dbg["outt"][:, m, :], outt[m][:])
            nc.sync.dma_start(
                dbg["vst"][:],
                vst[:].rearrange("p a b c -> p (a b c)"))


_PROGRAM = None


def _get_program():
    global _PROGRAM
    if _PROGRAM is None:
        nc = bacc.Bacc("TRN2", target_bir_lowering=False, debug=False)
        with tile.TileContext(nc) as tc:
            _emit(tc)
        nc.compile()
        _PROGRAM = nc
    return _PROGRAM


def make_in_maps(inputs):
    BF = ml_dtypes.bfloat16
    query, key, value = (np.asarray(inputs[k], np.float32)
                         for k in ("query", "key", "value"))
    Wq, Wk, Wv, Wo = (np.asarray(inputs[k], np.float32)
                      for k in ("Wq", "Wk", "Wv", "Wo"))
    bq, bk, bv = (np.asarray(inputs[k], np.float32) for k in ("bq", "bk", "bv"))

    def xprep(x, b):  # (2048, 1024) -> (128, N_SB, KI, SB) bf16
        t = x[b].T.reshape(KI, 128, N_SB, SB)
        return np.ascontiguousarray(t.transpose(1, 2, 0, 3)).astype(BF)

    def wslice(W, g):  # (1024, 256)-slice -> (128, 8, 256) bf16
        wt = W[g * O:(g + 1) * O].T  # (1024, 256)
        return np.ascontiguousarray(
            wt.reshape(KI, 128, O).transpose(1, 0, 2)
        ).astype(BF)

    in_maps = []
    for c in range(N_CORES):
        b, g = divmod(c, 4)
        sl = slice(g * O, (g + 1) * O)
        in_maps.append({
            "xqT": xprep(query, b),
            "xkT": xprep(key, b),
            "xvT": xprep(value, b),
            "wq": wslice(Wq, g),
            "wk": wslice(Wk, g),
            "wv": wslice(Wv, g),
            "wo": np.ascontiguousarray(
                Wo[:, sl].T.reshape(2, 128, H_DIM).transpose(1, 0, 2)
            ).astype(BF),
            "bq": np.ascontiguousarray(bq[sl].reshape(2, 128).T),
            "bk": np.ascontiguousarray(bk[sl].reshape(2, 128).T),
            "bv": np.ascontiguousarray(bv[sl].reshape(1, O)).astype(BF),
        })
    return in_maps


def kernel(query, key, value, Wq, bq, Wk, bk, Wv, bv, Wo, bo, _trace=False):
    bo = np.asarray(bo, np.float32)
    in_maps = make_in_maps(dict(
        query=query, key=key, value=value, Wq=Wq, Wk=Wk, Wv=Wv, Wo=Wo,
        bq=bq, bk=bk, bv=bv,
    ))

    nc = _get_program()
    res = run_bass_kernel_spmd(
        nc, in_maps, core_ids=list(range(N_CORES)), trace=_trace
    )
    out = np.empty((B, S, H_DIM), np.float32)
    for b in range(B):
        acc = res.results[4 * b]["y"].astype(np.float32)
        for g in range(1, 4):
            acc += res.results[4 * b + g]["y"].astype(np.float32)
        out[b] = acc + bo
    if _trace:
        kernel.last_result = res
    return out
